# revision 1
# baseline (speedup 1.0000x reference)
"""Bilinear grid_sample (zeros padding, align_corners=False) Bass kernel for TRN2.

Per-core problem: x [64, H*W] f32 (NCHW flattened), gxy [128, 2*NT] f32
(host-transposed grid planes; cols 0:NT = gx, NT:2NT = gy, where plane[p, t]
= grid[t*128 + p]), out [64, H*W] f32.

Strategy:
  1. Build a "vertical pair" gather table TBL [HW+W+2, 128] fp16 in DRAM:
       TBL[r, 0:64]   = x_nhwc[r - W - 1]   (zeros outside [0, HW))
       TBL[r, 64:128] = x_nhwc[r - 1]       (zeros outside [0, HW))
     built by transposing x tiles on TensorE ([64, 128] -> [128, 64]),
     casting f32->fp16 on DVE, and writing each staged [128, 4, 64] twice
     (lower half at rows 512c+1, upper half at rows 512c+W+1).
  2. Per pixel compute r = (clamp(y0,-1,H-1)+1)*W + clamp(x0,-1,W-1) + 1.
     One indirect-DMA gather of 2 consecutive rows (512 B) per pixel fetches
     all 4 bilinear taps for all 64 channels:
       [r, 0:64]=tap(y0,x0) [r, 64:128]=tap(y1,x0)
       [r+1, 0:64]=tap(y0,x1) [r+1, 64:128]=tap(y1,x1)
  3. Weighted sum on DVE (weights premasked for zeros padding), TensorE
     transpose back to [ch, px], ScalarE PSUM-evict cast fp16->f32, DMA out.
"""

from contextlib import ExitStack

import numpy as np

import concourse.bass as bass
import concourse.tile as tile
from concourse import mybir
from concourse.masks import make_identity

F32 = mybir.dt.float32
F16 = mybir.dt.float16
I32 = mybir.dt.int32
MUL = mybir.AluOpType.mult
ADD = mybir.AluOpType.add
SUB = mybir.AluOpType.subtract
MAX = mybir.AluOpType.max
MIN = mybir.AluOpType.min
GE = mybir.AluOpType.is_ge
LE = mybir.AluOpType.is_le
GT = mybir.AluOpType.is_gt


def gs_body(ctx: ExitStack, tc: tile.TileContext, out_ap, x_ap, gxy_ap, *,
            H=256, W=256, K=16):
    nc = tc.nc
    C = 64
    HW = H * W
    NT = HW // 128            # pixel tiles of 128 (weight-plane columns)
    assert HW % 512 == 0
    NCHUNK = HW // 512        # build chunks of 512 px
    NGATHER = NT // K         # gather iters, K tiles each
    TBL_ROWS = HW + W + 2

    tbl = nc.dram_tensor("tbl", [TBL_ROWS, 2 * C], F16, kind="Internal").ap()

    persist = ctx.enter_context(tc.tile_pool(name="persist", bufs=1))
    loadp = ctx.enter_context(tc.tile_pool(name="loadp", bufs=3))
    psumb = ctx.enter_context(tc.tile_pool(name="psumb", bufs=2, space="PSUM"))
    stage = ctx.enter_context(tc.tile_pool(name="stage", bufs=3))
    gath = ctx.enter_context(tc.tile_pool(name="gath", bufs=2))
    accp = ctx.enter_context(tc.tile_pool(name="accp", bufs=2))
    psumo = ctx.enter_context(tc.tile_pool(name="psumo", bufs=2, space="PSUM"))
    outp = ctx.enter_context(tc.tile_pool(name="outp", bufs=3))

    ident32 = persist.tile([128, 128], F32)
    make_identity(nc, ident32[:])
    ident16 = persist.tile([128, 128], F16)
    make_identity(nc, ident16[:])

    # ---------------- prologue: grid -> weights + gather indices ----------
    g_sb = persist.tile([128, 2 * NT], F32)
    nc.sync.dma_start(g_sb[:], gxy_ap[:])

    def axis_prep(gsl, size, ax):
        """Return (frac t, wm0=(1-t)*valid0, wm1=t*valid1, clamped floor)."""
        def ptile(dt, name):
            return persist.tile([128, NT], dt, name=f"{name}_{ax}",
                                tag=f"{name}_{ax}")
        v = ptile(F32, "v")
        # unnormalize: ((g+1)*size - 1)/2 = g*(size/2) + (size-1)/2
        nc.vector.tensor_scalar(v[:], gsl, size / 2.0, (size - 1) / 2.0, MUL, ADD)
        vi = ptile(I32, "vi")
        nc.vector.tensor_copy(vi[:], v[:])          # cast, rounding unknown
        vf = ptile(F32, "vf")
        nc.vector.tensor_copy(vf[:], vi[:])         # exact back-cast
        adj = ptile(F32, "adj")
        nc.vector.tensor_tensor(adj[:], vf[:], v[:], op=GT)  # 1.0 if vf > v
        nc.vector.tensor_tensor(vf[:], vf[:], adj[:], op=SUB)  # floor(v)
        t = ptile(F32, "t")
        nc.vector.tensor_tensor(t[:], v[:], vf[:], op=SUB)     # frac in [0,1)
        m0a = ptile(F32, "m0a")
        nc.vector.tensor_scalar(m0a[:], vf[:], 0.0, None, GE)
        m0b = ptile(F32, "m0b")
        nc.vector.tensor_scalar(m0b[:], vf[:], size - 1.0, None, LE)
        nc.vector.tensor_tensor(m0a[:], m0a[:], m0b[:], op=MUL)  # valid0
        m1a = ptile(F32, "m1a")
        nc.vector.tensor_scalar(m1a[:], vf[:], -1.0, None, GE)
        m1b = ptile(F32, "m1b")
        nc.vector.tensor_scalar(m1b[:], vf[:], size - 2.0, None, LE)
        nc.vector.tensor_tensor(m1a[:], m1a[:], m1b[:], op=MUL)  # valid1
        # wm0 = (1 - t) * valid0 ; wm1 = t * valid1
        wm0 = ptile(F32, "wm0")
        nc.vector.tensor_scalar(wm0[:], t[:], -1.0, 1.0, MUL, ADD)
        nc.vector.tensor_tensor(wm0[:], wm0[:], m0a[:], op=MUL)
        nc.vector.tensor_tensor(t[:], t[:], m1a[:], op=MUL)      # t <- wm1
        # clamped floor for addressing
        nc.vector.tensor_scalar(vf[:], vf[:], -1.0, size - 1.0, MAX, MIN)
        return wm0, t, vf

    wx0, wx1, xc = axis_prep(g_sb[:, 0:NT], float(W), "x")
    wy0, wy1, yc = axis_prep(g_sb[:, NT:2 * NT], float(H), "y")

    # combined weights, interleaved [p, (k t)] fp16 with t in (00,10,01,11)
    # order matching gathered layout [r:upper, r:lower, r+1:upper, r+1:lower]
    wcomb = persist.tile([128, NT * 4], F16)
    wv = wcomb[:].rearrange("p (k t) -> p k t", t=4)
    wtmp = persist.tile([128, NT], F32)
    for ti, (wy, wx) in enumerate(((wy0, wx0), (wy1, wx0), (wy0, wx1), (wy1, wx1))):
        nc.vector.tensor_tensor(wtmp[:], wy[:], wx[:], op=MUL)
        nc.vector.tensor_copy(wv[:, :, ti:ti + 1].squeeze(2), wtmp[:])

    # gather row index r = yc*W + xc + (W + 1), exact small ints in f32
    rf = persist.tile([128, NT], F32)
    nc.vector.tensor_scalar(rf[:], yc[:], float(W), W + 1.0, MUL, ADD)
    nc.vector.tensor_tensor(rf[:], rf[:], xc[:], op=ADD)
    idx = persist.tile([128, NT], I32)
    nc.vector.tensor_copy(idx[:], rf[:])

    # ---------------- zero the table edge rows ----------------------------
    zero_sb = persist.tile([128, 2 * C], F16)
    nc.gpsimd.memset(zero_sb[:], 0.0)

    def zero_rows(r0, r1):
        n = r1 - r0
        while n > 0:
            step = min(n, 128)
            nc.sync.dma_start(tbl[r0:r0 + step, :], zero_sb[0:step, :])
            r0 += step
            n -= step

    zero_rows(0, W + 1)                 # head: covers unused uppers + row 0
    zero_rows(HW + 1, HW + W + 2)       # tail: unused lowers + final row

    # ---------------- build the gather table ------------------------------
    for c in range(NCHUNK):
        xs = loadp.tile([C, 512], F32)
        nc.sync.dma_start(xs[:], x_ap[:, 512 * c:512 * (c + 1)])
        pt = psumb.tile([128, 4 * C], F32)
        for j in range(4):
            nc.tensor.transpose(pt[:, C * j:C * (j + 1)],
                                xs[:, 128 * j:128 * (j + 1)],
                                ident32[0:C, 0:C])
        st = stage.tile([128, 4 * C], F16)
        nc.vector.tensor_copy(st[:], pt[:])
        # lower halves at rows 512c+1, upper halves W rows later
        lo = tbl[512 * c + 1:512 * c + 513, C:2 * C]
        up = tbl[512 * c + W + 1:512 * c + W + 513, 0:C]
        stv = st[:].rearrange("p (j ch) -> p j ch", j=4)
        nc.sync.dma_start(lo.rearrange("(j p) ch -> p j ch", j=4), stv)
        nc.sync.dma_start(up.rearrange("(j p) ch -> p j ch", j=4), stv)

    # ---------------- gather + weighted sum + transpose out ---------------
    for g in range(NGATHER):
        gb = gath.tile([128, K * 4 * C], F16)
        # HW indirect DMA consumes ONE offset per partition and fetches a
        # contiguous [free_size] block, so issue one gather per 128-px tile.
        for k in range(K):
            nc.gpsimd.indirect_dma_start(
                out=gb[:, 4 * C * k:4 * C * (k + 1)],
                out_offset=None,
                in_=tbl[:],
                in_offset=bass.IndirectOffsetOnAxis(
                    ap=idx[:, K * g + k:K * g + k + 1], axis=0),
            )
        gb4 = gb[:].rearrange("p (k t ch) -> p k t ch", k=K, t=4)
        wsl = wcomb[:, 4 * K * g:4 * K * (g + 1)]
        wb = wsl.rearrange("p (k t) -> p k t", t=4)
        acc = accp.tile([128, K * C], F16)
        tmp = accp.tile([128, K * C], F16)
        accv = acc[:].rearrange("p (k ch) -> p k ch", k=K)
        tmpv = tmp[:].rearrange("p (k ch) -> p k ch", k=K)
        for ti in range(4):
            dst = accv if ti == 0 else tmpv
            nc.vector.tensor_tensor(
                dst,
                gb4[:, :, ti:ti + 1, :].squeeze(2),
                wb[:, :, ti:ti + 1].to_broadcast([128, K, C]),
                op=MUL,
            )
            if ti > 0:
                nc.vector.tensor_tensor(accv, accv, tmpv, op=ADD)
        po = psumo.tile([C, K * 128], F16)
        for t in range(K):
            nc.tensor.transpose(po[:, 128 * t:128 * (t + 1)],
                                acc[:, C * t:C * (t + 1)],
                                ident16[:])
        ob = outp.tile([C, K * 128], F32)
        nc.scalar.activation(ob[:], po[:], mybir.ActivationFunctionType.Copy)
        nc.sync.dma_start(out_ap[:, 128 * K * g:128 * K * (g + 1)], ob[:])


def host_prep_gxy(grid_flat):
    """grid_flat [HW, 2] f32 -> [128, 2*NT] f32 (gx plane | gy plane)."""
    HW = grid_flat.shape[0]
    NT = HW // 128
    g = grid_flat.reshape(NT, 128, 2)
    return np.ascontiguousarray(
        np.concatenate([g[:, :, 0].T, g[:, :, 1].T], axis=1))




# ----------------------------------------------------------------------------
# self-contained kernel entry point
# ----------------------------------------------------------------------------
import concourse.bacc as bacc
from concourse.bass_utils import run_bass_kernel_spmd

N_CORES = 8
H = W = 256
C = 64
HW = H * W
K = 16

_NC = None
LAST_RESULT = None


def _build_nc():
    global _NC
    if _NC is not None:
        return _NC
    nc = bacc.Bacc("TRN2", target_bir_lowering=False, debug=False)
    x = nc.dram_tensor("x", [C, HW], F32, kind="ExternalInput").ap()
    gxy = nc.dram_tensor("gxy", [128, 2 * (HW // 128)], F32,
                         kind="ExternalInput").ap()
    out = nc.dram_tensor("out", [C, HW], F32, kind="ExternalOutput").ap()
    with tile.TileContext(nc) as tc, ExitStack() as ctx:
        gs_body(ctx, tc, out, x, gxy, H=H, W=W, K=K)
    nc.compile()
    _NC = nc
    return nc


def kernel(x, grid, trace=False):
    global LAST_RESULT
    x = np.asarray(x, dtype=np.float32)
    grid = np.asarray(grid, dtype=np.float32)
    assert x.shape == (N_CORES, C, H, W) and grid.shape == (N_CORES, H, W, 2)
    nc = _build_nc()
    in_maps = []
    for n in range(N_CORES):
        in_maps.append({
            "x": np.ascontiguousarray(x[n].reshape(C, HW)),
            "gxy": host_prep_gxy(grid[n].reshape(HW, 2)),
        })
    res = run_bass_kernel_spmd(nc, in_maps, core_ids=list(range(N_CORES)),
                               trace=trace)
    LAST_RESULT = res
    out = np.stack([m["out"] for m in res.results])
    return out.reshape(N_CORES, C, H, W)

